# revision 22
# baseline (speedup 1.0000x reference)
"""Additive (Bahdanau) attention TRN2 kernel.

Problem shapes (hardcoded): B=4, QL=256, KL=1024, QD=256, KD=512, VD=512.
Sharding: 8 cores = (batch, q-half); each core computes 128 query rows
against that batch's (compacted) keys.

Key insight: ~half the keys are masked by key_pad_mask, and the reference
gives those attention weight exactly 0.0 (exp(-1e12 - max) underflows).
So the host gathers only the valid keys per batch, the device computes
attention over the compacted key set, and the host scatters weights back.

Device algorithm per core (q = one of 128 query rows, e = 256 feature dim,
k = KC compacted keys):
  mKT[e, k] = (Wk @ K_c^T)[e, k] + bk[e]           (TensorE + DVE evac)
  mQT[e, q] = (Wq @ Q^T)[e, q] + bq[e]             (TensorE + DVE evac)
  per q:  T[e, k] = tanh(mKT[e, k] + mQT[e, q])    (ONE ScalarE activation
                                                    per 128-e chunk: the mQ
                                                    column is the per-partition
                                                    bias)
          scores_psum[:, k] = sum_e wV[e] T[e, k]  (TensorE matmul, stationary
                                                    operand = wV replicated to
                                                    128 cols -> every PSUM
                                                    partition holds the same
                                                    scores row)
          scores[q, k] = scores_psum[q, k] + mask[k]  (1-partition DVE add:
                                                    partition q reads its own
                                                    copy; mask = bV on valid
                                                    cols, -1e12 on padding)
  softmax over k (reduce_max negate -> Exp activation with accum_out sum ->
  reciprocal -> scale), ctx = weights @ V via PE transpose + matmul.
"""

import os

import numpy as np

B, QL, KL = 4, 256, 1024
QD, KD, VD = 256, 512, 512
N_CORES = 8
QS = QL // 2  # 128 queries per core
NEG = -1.0e12

# Reduce dtype for the big e-reduction + ctx matmuls: float32r streams at
# 1 col/cycle (vs 4 for float32) on the PE.
_F32R_REDUCE = os.environ.get("KERNEL_F32R", "1") == "1"
_BF16_REDUCE = os.environ.get("KERNEL_BF16", "0") == "1"

_prog_cache: dict[tuple, object] = {}

# Stash of the most recent BassKernelResults (for test harnesses to read
# exec_time_ns when BASS_TRACE=1).
LAST_RESULTS = None


def _build(KC: int):
    import concourse.bacc as bacc
    from concourse import mybir
    from concourse.tile import TileContext

    f32 = mybir.dt.float32
    f32r = mybir.dt.float32r if _F32R_REDUCE else mybir.dt.float32
    rdt = mybir.dt.bfloat16 if _BF16_REDUCE else f32r
    MW = 128 if _BF16_REDUCE else 1  # stationary-operand column count
    AF = mybir.ActivationFunctionType

    NE = QD // 128  # 2 e-chunks
    ND_Q = QD // 128  # 2 d-chunks for Wq
    ND_K = KD // 128  # 4 d-chunks for Wk
    # N-splits of KC for matmul moving operand (<=512 each)
    nsplits = [(0, min(KC, 512))]
    if KC > 512:
        nsplits.append((512, KC - 512))
    # k chunks (<=128) for the ctx matmul contraction
    kchunks = []
    k0 = 0
    while k0 < KC:
        kchunks.append((k0, min(128, KC - k0)))
        k0 += 128

    nc = bacc.Bacc()

    qt = nc.dram_tensor("qt", [QD, QS], f32r, kind="ExternalInput")
    kt = nc.dram_tensor("kt", [KD, KC], f32r, kind="ExternalInput")
    vc = nc.dram_tensor("vc", [KC, VD], f32r, kind="ExternalInput")
    wqt = nc.dram_tensor("wqt", [QD, QD], f32r, kind="ExternalInput")  # Wq.T
    wkt = nc.dram_tensor("wkt", [KD, QD], f32r, kind="ExternalInput")  # Wk.T
    bq2 = nc.dram_tensor("bq2", [128, NE], f32, kind="ExternalInput")
    bk2 = nc.dram_tensor("bk2", [128, NE], f32, kind="ExternalInput")
    wvr = nc.dram_tensor("wvr", [128, NE, MW], rdt, kind="ExternalInput")
    maskv = nc.dram_tensor("maskv", [1, KC], f32, kind="ExternalInput")
    ident = nc.dram_tensor("ident", [128, 128], f32, kind="ExternalInput")

    w_out = nc.dram_tensor("w_out", [QS, KC], f32, kind="ExternalOutput")
    ctx_out = nc.dram_tensor("ctx_out", [QS, VD], f32, kind="ExternalOutput")

    with TileContext(nc) as tc:
        with (
            tc.tile_pool(name="consts", bufs=1) as consts,
            tc.tile_pool(name="tanh", bufs=2) as tanhp,
            tc.tile_pool(name="stage", bufs=2) as stagep,
            tc.tile_pool(name="psum", bufs=4, space="PSUM") as psump,
        ):
            # ---- PE early start: a few dummy matmuls on a zeroed tile so
            # the PE sequencer's instruction fetch + pipeline startup happen
            # during the DMA phase instead of stalling the first real matmul.
            warm_sb = consts.tile([128, 16], f32)
            nc.vector.memset(warm_sb, 0.0)
            warm_ps = psump.tile([1, 16], f32, tag="p")
            for _ in range(4):
                nc.tensor.matmul(
                    warm_ps[0:1, :],
                    lhsT=warm_sb[:, 0:1].bitcast(f32r),
                    rhs=warm_sb[:, :].bitcast(f32r),
                    start=True, stop=True,
                )

            # ---- load constants / inputs ----
            # DMA issue costs ~0.6us per dma_start on a queue; the first
            # matmuls need kt+wkt, so those issue first, split across the
            # sync and gpsimd queues.  V / identity / mask are not needed
            # until much later and issue lazily.
            kt_sb = consts.tile([128, ND_K, KC], f32r)
            wkt_sb = consts.tile([128, ND_K, QD], f32r)
            for c in range(ND_K):
                nc.sync.dma_start(out=kt_sb[:, c, :], in_=kt[c * 128 : (c + 1) * 128, :])
                nc.gpsimd.dma_start(out=wkt_sb[:, c, :], in_=wkt[c * 128 : (c + 1) * 128, :])
            qt_sb = consts.tile([128, ND_Q, QS], f32r)
            wqt_sb = consts.tile([128, ND_Q, QD], f32r)
            for c in range(ND_Q):
                nc.scalar.dma_start(out=qt_sb[:, c, :], in_=qt[c * 128 : (c + 1) * 128, :])
                nc.scalar.dma_start(out=wqt_sb[:, c, :], in_=wqt[c * 128 : (c + 1) * 128, :])
            bq_sb = consts.tile([128, NE], f32)
            nc.scalar.dma_start(out=bq_sb, in_=bq2[:, :])
            bk_sb = consts.tile([128, NE], f32)
            nc.gpsimd.dma_start(out=bk_sb, in_=bk2[:, :])
            wv_sb = consts.tile([128, NE, MW], rdt)
            nc.gpsimd.dma_start(out=wv_sb, in_=wvr[:, :, :])
            mask_bc = consts.tile([128, KC], f32)
            nc.gpsimd.dma_start(out=mask_bc, in_=maskv[0:1, :].to_broadcast([128, KC]))
            ident_sb = consts.tile([128, 128], f32)
            v_sb = consts.tile([128, len(kchunks), VD], f32r)

            # ---- mapped_Q^T [e, q] and mapped_K^T [e, k] (f32r matmuls) ----
            mkt_sb = consts.tile([128, NE, KC], f32)
            mqt_sb = consts.tile([128, NE, QS], f32)

            def emit_mkt(ce):
                for n0, nn in nsplits:
                    ps = psump.tile([128, 512], f32, tag="p", name=f"psk{ce}_{n0}")
                    for cd in range(ND_K):
                        nc.tensor.matmul(
                            ps[:, :nn],
                            lhsT=wkt_sb[:, cd, ce * 128 : (ce + 1) * 128],
                            rhs=kt_sb[:, cd, n0 : n0 + nn],
                            start=(cd == 0),
                            stop=(cd == ND_K - 1),
                        )
                    nc.vector.tensor_scalar_add(
                        mkt_sb[:, ce, n0 : n0 + nn], ps[:, :nn], bk_sb[:, ce : ce + 1]
                    )

            def emit_mqt(ce):
                ps = psump.tile([128, 512], f32, tag="p", name=f"psq{ce}")
                for cd in range(ND_Q):
                    nc.tensor.matmul(
                        ps[:, :QS],
                        lhsT=wqt_sb[:, cd, ce * 128 : (ce + 1) * 128],
                        rhs=qt_sb[:, cd, :],
                        start=(cd == 0),
                        stop=(cd == ND_Q - 1),
                    )
                nc.vector.tensor_scalar_add(mqt_sb[:, ce, :], ps[:, :QS], bq_sb[:, ce : ce + 1])

            # first ACTIVATE only needs mkt[ce0] and mqt[ce0]; mkt[ce1]
            # overlaps with the start of the main loop (PE has enough slack
            # per query to absorb it).
            emit_mkt(0)
            emit_mqt(0)
            emit_mqt(1)
            emit_mkt(1)

            # ---- main loop: scores, 2 queries per group ----
            # tanh tiles are grouped 2 queries wide so the ScalarE pays the
            # pool-reuse wait once per 2 ACTIVATEs, and the wv stationary
            # operand is reloaded once per (group, e-chunk) instead of per
            # matmul.  Each q's scores row lands on PSUM partition 0 (M=1
            # matmul); DVE copies it into a partition-0 staging tile of G
            # rows which is scatter-DMA'd (SBUF->SBUF) into [q, k] layout.
            G2 = 8   # queries per tanh tile group
            GP = 4   # queries whose matmuls run per stationary-operand pass
            G = 8    # queries per scatter chunk
            scores_sb = consts.tile([128, KC], f32)
            stage = None
            for g in range(QS // G2):
                ths = []
                for ce in range(NE):
                    th_grp = tanhp.tile([128, G2, KC], rdt, tag=f"tanh{ce}", name=f"th{g}_{ce}")
                    ths.append(th_grp)
                for j in range(G2):
                    q = g * G2 + j
                    for ce in range(NE):
                        nc.scalar.activation(
                            ths[ce][:, j, :],
                            mkt_sb[:, ce, :],
                            AF.Tanh,
                            bias=mqt_sb[:, ce, q : q + 1],
                            scale=1.0,
                        )
                for jp in range(G2 // GP):
                    pss = []
                    for j2 in range(GP):
                        ps_j = psump.tile([MW, KC], f32, tag="p", name=f"ps{g}_{jp}_{j2}")
                        pss.append(ps_j)
                    for ce in range(NE):
                        for j2 in range(GP):
                            j = jp * GP + j2
                            for n0, nn in nsplits:
                                nc.tensor.matmul(
                                    pss[j2][0:MW, n0 : n0 + nn],
                                    lhsT=wv_sb[:, ce, :],
                                    rhs=ths[ce][:, j, n0 : n0 + nn],
                                    start=(ce == 0),
                                    stop=(ce == NE - 1),
                                )
                    for j2 in range(GP):
                        q = g * G2 + jp * GP + j2
                        if stage is None:
                            stage = stagep.tile([1, G, KC], f32, tag="stage")
                        # evacuate + apply padding mask/bV in one op (all of
                        # these rows live on partition 0)
                        nc.vector.tensor_add(
                            stage[0:1, q % G, :], pss[j2][0:1, :], mask_bc[0:1, :]
                        )
                        if q % G == G - 1:
                            nc.sync.dma_start(
                                out=scores_sb[q - G + 1 : q + 1, :],
                                in_=stage[0:1, :, :],
                            )
                            stage = None

            # ---- softmax ----
            # no max-subtraction: |scores| <= sum|wV| + |bV| < 26 (tanh in
            # [-1,1]), so exp cannot overflow fp32; padded columns are -1e12
            # and exp to exactly 0.  Matches the reference softmax exactly up
            # to fp32 rounding.
            p_sb = consts.tile([128, KC], f32)
            sum_sb = consts.tile([128, 1], f32)
            nc.scalar.activation(
                p_sb, scores_sb, AF.Exp, bias=0.0, scale=1.0, accum_out=sum_sb
            )
            recip = consts.tile([128, 1], f32)
            nc.vector.reciprocal(recip, sum_sb)
            wout_sb = consts.tile([128, KC], f32)
            nc.vector.tensor_scalar_mul(wout_sb, p_sb, recip[:, 0:1])
            nc.sync.dma_start(out=w_out[:, :], in_=wout_sb)

            # ---- ctx = softmax(scores) @ V ----
            nc.gpsimd.dma_start(out=ident_sb, in_=ident[:, :])
            for t, (c0, cn) in enumerate(kchunks):
                nc.gpsimd.dma_start(out=v_sb[:cn, t, :], in_=vc[c0 : c0 + cn, :])
            pT_sb = consts.tile([128, len(kchunks), 128], f32r)
            for t, (c0, cn) in enumerate(kchunks):
                pst = psump.tile([128, 512], f32, tag="p")
                nc.tensor.transpose(pst[:cn, :128], p_sb[:, c0 : c0 + cn], ident_sb)
                nc.vector.tensor_copy(pT_sb[:cn, t, :], pst[:cn, :128])
            ps_ctx = psump.tile([128, 512], f32, tag="p")
            for t, (c0, cn) in enumerate(kchunks):
                nc.tensor.matmul(
                    ps_ctx[:, :VD],
                    lhsT=pT_sb[:cn, t, :],
                    rhs=v_sb[:cn, t, :],
                    start=(t == 0),
                    stop=(t == len(kchunks) - 1),
                )
            ctx_sb = consts.tile([128, VD], f32)
            nc.vector.tensor_scalar_mul(ctx_sb, ps_ctx[:, :VD], recip[:, 0:1])
            nc.sync.dma_start(out=ctx_out[:, :], in_=ctx_sb)

    nc.compile()
    return nc


def _ensure_ntff_hook_module():
    """bass_utils' trace path (BASS_TRACE=1 under axon) imports
    antenv.axon_hooks unconditionally; some images lack that submodule.
    Provide a no-op implementation so tracing degrades instead of
    crashing.  Never overwrites a real or already-installed module."""
    import sys
    import types

    try:
        import antenv.axon_hooks  # noqa: F401
        return
    except ImportError:
        pass
    mod = types.ModuleType("antenv.axon_hooks")
    state = {"hook": None}
    mod.set_axon_ntff_profile_hook = lambda h: state.__setitem__("hook", h)
    mod.get_axon_ntff_profile_hook = lambda: state["hook"]
    sys.modules["antenv.axon_hooks"] = mod


def _kernel_numpy(Q, K, V, key_pad_mask, Wq, bq, Wk, bk, wV, bV):
    """Reference-equivalent numpy fallback for degenerate inputs."""
    mq = np.einsum("bqd,ed->bqe", Q, Wq) + bq
    mk = np.einsum("bkd,ed->bke", K, Wk) + bk
    scores = np.einsum(
        "bqke,e->bqk", np.tanh(mq[:, :, None, :] + mk[:, None, :, :]), wV.reshape(-1)
    ) + bV.reshape(-1)[0]
    scores = np.where(key_pad_mask[:, None, :], NEG, scores).astype(np.float32)
    e = np.exp(scores - scores.max(-1, keepdims=True))
    w = (e / e.sum(-1, keepdims=True)).astype(np.float32)
    ctx = np.einsum("bqk,bkv->bqv", w, V).astype(np.float32)
    return ctx, w


def kernel(Q, K, V, key_pad_mask, Wq, bq, Wk, bk, wV, bV):
    global LAST_RESULTS
    _ensure_ntff_hook_module()
    from concourse.bass_utils import run_bass_kernel_spmd

    Q = np.asarray(Q, np.float32)
    K = np.asarray(K, np.float32)
    V = np.asarray(V, np.float32)
    key_pad_mask = np.asarray(key_pad_mask, bool)
    Wq = np.asarray(Wq, np.float32)
    bq = np.asarray(bq, np.float32)
    Wk = np.asarray(Wk, np.float32)
    bk = np.asarray(bk, np.float32)
    wV = np.asarray(wV, np.float32).reshape(-1)  # [QD]
    bV = np.asarray(bV, np.float32).reshape(-1)  # [1]

    if Q.shape != (B, QL, QD) or K.shape != (B, KL, KD) or V.shape != (B, KL, VD):
        return _kernel_numpy(Q, K, V, key_pad_mask, Wq, bq, Wk, bk, wV, bV)

    valid = [np.nonzero(~key_pad_mask[b])[0] for b in range(B)]
    counts = [len(v) for v in valid]
    if min(counts) == 0:
        # a fully-masked batch makes the reference softmax uniform over all
        # keys; the compacted device path cannot represent that.
        return _kernel_numpy(Q, K, V, key_pad_mask, Wq, bq, Wk, bk, wV, bV)
    maxc = max(counts)
    KC = max(512, -(-maxc // 8) * 8)

    key = (KC,)
    if key not in _prog_cache:
        _prog_cache[key] = _build(KC)
    nc = _prog_cache[key]

    NE = QD // 128
    wqt = np.ascontiguousarray(Wq.T)  # [d, e]
    wkt = np.ascontiguousarray(Wk.T)  # [d, e]
    bq2 = np.ascontiguousarray(bq.reshape(NE, 128).T)  # [128, NE]
    bk2 = np.ascontiguousarray(bk.reshape(NE, 128).T)
    # wvr[p, c, m] = wV[c*128 + p]
    MW = 128 if _BF16_REDUCE else 1
    wvr = np.ascontiguousarray(np.broadcast_to(wV.reshape(NE, 128).T[:, :, None], (128, NE, MW)))
    if _BF16_REDUCE:
        import ml_dtypes
        wvr = np.ascontiguousarray(wvr.astype(ml_dtypes.bfloat16))
    identity = np.eye(128, dtype=np.float32)

    in_maps = []
    for c in range(N_CORES):
        b, half = divmod(c, 2)
        idx = valid[b]
        cnt = counts[b]
        qt_c = np.ascontiguousarray(Q[b, half * QS : (half + 1) * QS, :].T)  # [QD, QS]
        kt_c = np.zeros((KD, KC), np.float32)
        kt_c[:, :cnt] = K[b, idx, :].T
        vc_c = np.zeros((KC, VD), np.float32)
        vc_c[:cnt] = V[b, idx, :]
        maskv = np.full((1, KC), NEG, np.float32)
        maskv[0, :cnt] = bV[0]
        in_maps.append(
            {
                "qt": qt_c,
                "kt": kt_c,
                "vc": vc_c,
                "wqt": wqt,
                "wkt": wkt,
                "bq2": bq2,
                "bk2": bk2,
                "wvr": wvr,
                "maskv": maskv,
                "ident": identity,
            }
        )

    res = run_bass_kernel_spmd(nc, in_maps, core_ids=list(range(N_CORES)))
    LAST_RESULTS = res

    attn_ctx = np.zeros((B, QL, VD), np.float32)
    attn_w = np.zeros((B, QL, KL), np.float32)
    for c in range(N_CORES):
        b, half = divmod(c, 2)
        idx = valid[b]
        cnt = counts[b]
        out = res.results[c]
        attn_ctx[b, half * QS : (half + 1) * QS, :] = out["ctx_out"]
        attn_w[b, half * QS : (half + 1) * QS][:, idx] = out["w_out"][:, :cnt]
    return attn_ctx, attn_w


# revision 23
# speedup vs baseline: 1.0063x; 1.0063x over previous
"""Additive (Bahdanau) attention TRN2 kernel.

Problem shapes (hardcoded): B=4, QL=256, KL=1024, QD=256, KD=512, VD=512.
Sharding: 8 cores = (batch, q-half); each core computes 128 query rows
against that batch's (compacted) keys.

Key insight: ~half the keys are masked by key_pad_mask, and the reference
gives those attention weight exactly 0.0 (exp(-1e12 - max) underflows).
So the host gathers only the valid keys per batch, the device computes
attention over the compacted key set, and the host scatters weights back.

Device algorithm per core (q = one of 128 query rows, e = 256 feature dim,
k = KC compacted keys):
  mKT[e, k] = (Wk @ K_c^T)[e, k] + bk[e]           (TensorE + DVE evac)
  mQT[e, q] = (Wq @ Q^T)[e, q] + bq[e]             (TensorE + DVE evac)
  per q:  T[e, k] = tanh(mKT[e, k] + mQT[e, q])    (ONE ScalarE activation
                                                    per 128-e chunk: the mQ
                                                    column is the per-partition
                                                    bias)
          scores_psum[:, k] = sum_e wV[e] T[e, k]  (TensorE matmul, stationary
                                                    operand = wV replicated to
                                                    128 cols -> every PSUM
                                                    partition holds the same
                                                    scores row)
          scores[q, k] = scores_psum[q, k] + mask[k]  (1-partition DVE add:
                                                    partition q reads its own
                                                    copy; mask = bV on valid
                                                    cols, -1e12 on padding)
  softmax over k (reduce_max negate -> Exp activation with accum_out sum ->
  reciprocal -> scale), ctx = weights @ V via PE transpose + matmul.
"""

import os

import numpy as np

B, QL, KL = 4, 256, 1024
QD, KD, VD = 256, 512, 512
N_CORES = 8
QS = QL // 2  # 128 queries per core
NEG = -1.0e12

# Reduce dtype for the big e-reduction + ctx matmuls: float32r streams at
# 1 col/cycle (vs 4 for float32) on the PE.
_F32R_REDUCE = os.environ.get("KERNEL_F32R", "1") == "1"
_BF16_REDUCE = os.environ.get("KERNEL_BF16", "0") == "1"

_prog_cache: dict[tuple, object] = {}

# Stash of the most recent BassKernelResults (for test harnesses to read
# exec_time_ns when BASS_TRACE=1).
LAST_RESULTS = None


def _build(KC: int):
    import concourse.bacc as bacc
    from concourse import mybir
    from concourse.tile import TileContext

    f32 = mybir.dt.float32
    f16 = mybir.dt.float16
    f32r = mybir.dt.float32r if _F32R_REDUCE else mybir.dt.float32
    rdt = mybir.dt.bfloat16 if _BF16_REDUCE else f32r
    MW = 128 if _BF16_REDUCE else 1  # stationary-operand column count
    AF = mybir.ActivationFunctionType

    NE = QD // 128  # 2 e-chunks
    ND_Q = QD // 128  # 2 d-chunks for Wq
    ND_K = KD // 128  # 4 d-chunks for Wk
    # N-splits of KC for matmul moving operand (<=512 each)
    nsplits = [(0, min(KC, 512))]
    if KC > 512:
        nsplits.append((512, KC - 512))
    # k chunks (<=128) for the ctx matmul contraction
    kchunks = []
    k0 = 0
    while k0 < KC:
        kchunks.append((k0, min(128, KC - k0)))
        k0 += 128

    nc = bacc.Bacc()

    qt = nc.dram_tensor("qt", [QD, QS], f32r, kind="ExternalInput")
    kt = nc.dram_tensor("kt", [KD, KC], f16, kind="ExternalInput")
    vc = nc.dram_tensor("vc", [KC, VD], f32r, kind="ExternalInput")
    wqt = nc.dram_tensor("wqt", [QD, QD], f32r, kind="ExternalInput")  # Wq.T
    wkt = nc.dram_tensor("wkt", [KD, QD], f16, kind="ExternalInput")  # Wk.T
    bq2 = nc.dram_tensor("bq2", [128, NE], f32, kind="ExternalInput")
    bk2 = nc.dram_tensor("bk2", [128, NE], f32, kind="ExternalInput")
    wvr = nc.dram_tensor("wvr", [128, NE, MW], rdt, kind="ExternalInput")
    maskv = nc.dram_tensor("maskv", [1, KC], f32, kind="ExternalInput")
    ident = nc.dram_tensor("ident", [128, 128], f32, kind="ExternalInput")

    w_out = nc.dram_tensor("w_out", [QS, KC], f32, kind="ExternalOutput")
    ctx_out = nc.dram_tensor("ctx_out", [QS, VD], f32, kind="ExternalOutput")

    with TileContext(nc) as tc:
        with (
            tc.tile_pool(name="consts", bufs=1) as consts,
            tc.tile_pool(name="tanh", bufs=2) as tanhp,
            tc.tile_pool(name="stage", bufs=2) as stagep,
            tc.tile_pool(name="psum", bufs=4, space="PSUM") as psump,
        ):
            # ---- PE early start: a few dummy matmuls on a zeroed tile so
            # the PE sequencer's instruction fetch + pipeline startup happen
            # during the DMA phase instead of stalling the first real matmul.
            warm_sb = consts.tile([128, 16], f32)
            nc.vector.memset(warm_sb, 0.0)
            warm_ps = psump.tile([1, 16], f32, tag="p")
            for _ in range(4):
                nc.tensor.matmul(
                    warm_ps[0:1, :],
                    lhsT=warm_sb[:, 0:1].bitcast(f32r),
                    rhs=warm_sb[:, :].bitcast(f32r),
                    start=True, stop=True,
                )

            # ---- load constants / inputs ----
            # DMA issue costs ~0.6us per dma_start on a queue; the first
            # matmuls need kt+wkt, so those issue first, split across the
            # sync and gpsimd queues.  V / identity / mask are not needed
            # until much later and issue lazily.
            kt_sb = consts.tile([128, ND_K, KC], f16)
            wkt_sb = consts.tile([128, ND_K, QD], f16)
            for c in range(ND_K):
                nc.sync.dma_start(out=kt_sb[:, c, :], in_=kt[c * 128 : (c + 1) * 128, :])
                nc.gpsimd.dma_start(out=wkt_sb[:, c, :], in_=wkt[c * 128 : (c + 1) * 128, :])
            qt_sb = consts.tile([128, ND_Q, QS], f32r)
            wqt_sb = consts.tile([128, ND_Q, QD], f32r)
            for c in range(ND_Q):
                nc.scalar.dma_start(out=qt_sb[:, c, :], in_=qt[c * 128 : (c + 1) * 128, :])
                nc.scalar.dma_start(out=wqt_sb[:, c, :], in_=wqt[c * 128 : (c + 1) * 128, :])
            bq_sb = consts.tile([128, NE], f32)
            nc.scalar.dma_start(out=bq_sb, in_=bq2[:, :])
            bk_sb = consts.tile([128, NE], f32)
            nc.gpsimd.dma_start(out=bk_sb, in_=bk2[:, :])
            wv_sb = consts.tile([128, NE, MW], rdt)
            nc.gpsimd.dma_start(out=wv_sb, in_=wvr[:, :, :])
            mask_bc = consts.tile([128, KC], f32)
            nc.gpsimd.dma_start(out=mask_bc, in_=maskv[0:1, :].to_broadcast([128, KC]))
            ident_sb = consts.tile([128, 128], f32)
            v_sb = consts.tile([128, len(kchunks), VD], f32r)

            # ---- mapped_Q^T [e, q] and mapped_K^T [e, k] (f32r matmuls) ----
            mkt_sb = consts.tile([128, NE, KC], f32)
            mqt_sb = consts.tile([128, NE, QS], f32)

            def emit_mkt(ce):
                for n0, nn in nsplits:
                    ps = psump.tile([128, 512], f32, tag="p", name=f"psk{ce}_{n0}")
                    for cd in range(ND_K):
                        nc.tensor.matmul(
                            ps[:, :nn],
                            lhsT=wkt_sb[:, cd, ce * 128 : (ce + 1) * 128],
                            rhs=kt_sb[:, cd, n0 : n0 + nn],
                            start=(cd == 0),
                            stop=(cd == ND_K - 1),
                        )
                    nc.vector.tensor_scalar_add(
                        mkt_sb[:, ce, n0 : n0 + nn], ps[:, :nn], bk_sb[:, ce : ce + 1]
                    )

            def emit_mqt(ce):
                ps = psump.tile([128, 512], f32, tag="p", name=f"psq{ce}")
                for cd in range(ND_Q):
                    nc.tensor.matmul(
                        ps[:, :QS],
                        lhsT=wqt_sb[:, cd, ce * 128 : (ce + 1) * 128],
                        rhs=qt_sb[:, cd, :],
                        start=(cd == 0),
                        stop=(cd == ND_Q - 1),
                    )
                nc.vector.tensor_scalar_add(mqt_sb[:, ce, :], ps[:, :QS], bq_sb[:, ce : ce + 1])

            # first ACTIVATE only needs mkt[ce0] and mqt[ce0]; mkt[ce1]
            # overlaps with the start of the main loop (PE has enough slack
            # per query to absorb it).
            emit_mkt(0)
            emit_mqt(0)
            emit_mqt(1)
            emit_mkt(1)

            # ---- main loop: scores, 2 queries per group ----
            # tanh tiles are grouped 2 queries wide so the ScalarE pays the
            # pool-reuse wait once per 2 ACTIVATEs, and the wv stationary
            # operand is reloaded once per (group, e-chunk) instead of per
            # matmul.  Each q's scores row lands on PSUM partition 0 (M=1
            # matmul); DVE copies it into a partition-0 staging tile of G
            # rows which is scatter-DMA'd (SBUF->SBUF) into [q, k] layout.
            G2 = 8   # queries per tanh tile group
            GP = 4   # queries whose matmuls run per stationary-operand pass
            G = 8    # queries per scatter chunk
            scores_sb = consts.tile([128, KC], f32)
            stage = None
            for g in range(QS // G2):
                ths = []
                for ce in range(NE):
                    th_grp = tanhp.tile([128, G2, KC], rdt, tag=f"tanh{ce}", name=f"th{g}_{ce}")
                    ths.append(th_grp)
                for j in range(G2):
                    q = g * G2 + j
                    for ce in range(NE):
                        nc.scalar.activation(
                            ths[ce][:, j, :],
                            mkt_sb[:, ce, :],
                            AF.Tanh,
                            bias=mqt_sb[:, ce, q : q + 1],
                            scale=1.0,
                        )
                for jp in range(G2 // GP):
                    pss = []
                    for j2 in range(GP):
                        ps_j = psump.tile([MW, KC], f32, tag="p", name=f"ps{g}_{jp}_{j2}")
                        pss.append(ps_j)
                    for ce in range(NE):
                        for j2 in range(GP):
                            j = jp * GP + j2
                            for n0, nn in nsplits:
                                nc.tensor.matmul(
                                    pss[j2][0:MW, n0 : n0 + nn],
                                    lhsT=wv_sb[:, ce, :],
                                    rhs=ths[ce][:, j, n0 : n0 + nn],
                                    start=(ce == 0),
                                    stop=(ce == NE - 1),
                                )
                    for j2 in range(GP):
                        q = g * G2 + jp * GP + j2
                        if stage is None:
                            stage = stagep.tile([1, G, KC], f32, tag="stage")
                        # evacuate + apply padding mask/bV in one op (all of
                        # these rows live on partition 0)
                        nc.vector.tensor_add(
                            stage[0:1, q % G, :], pss[j2][0:1, :], mask_bc[0:1, :]
                        )
                        if q % G == G - 1:
                            nc.sync.dma_start(
                                out=scores_sb[q - G + 1 : q + 1, :],
                                in_=stage[0:1, :, :],
                            )
                            stage = None

            # ---- softmax ----
            # no max-subtraction: |scores| <= sum|wV| + |bV| < 26 (tanh in
            # [-1,1]), so exp cannot overflow fp32; padded columns are -1e12
            # and exp to exactly 0.  Matches the reference softmax exactly up
            # to fp32 rounding.
            p_sb = consts.tile([128, KC], f32)
            sum_sb = consts.tile([128, 1], f32)
            nc.scalar.activation(
                p_sb, scores_sb, AF.Exp, bias=0.0, scale=1.0, accum_out=sum_sb
            )
            recip = consts.tile([128, 1], f32)
            nc.vector.reciprocal(recip, sum_sb)
            wout_sb = consts.tile([128, KC], f32)
            nc.vector.tensor_scalar_mul(wout_sb, p_sb, recip[:, 0:1])
            nc.sync.dma_start(out=w_out[:, :], in_=wout_sb)

            # ---- ctx = softmax(scores) @ V ----
            nc.gpsimd.dma_start(out=ident_sb, in_=ident[:, :])
            for t, (c0, cn) in enumerate(kchunks):
                nc.gpsimd.dma_start(out=v_sb[:cn, t, :], in_=vc[c0 : c0 + cn, :])
            pT_sb = consts.tile([128, len(kchunks), 128], f32r)
            for t, (c0, cn) in enumerate(kchunks):
                pst = psump.tile([128, 512], f32, tag="p")
                nc.tensor.transpose(pst[:cn, :128], p_sb[:, c0 : c0 + cn], ident_sb)
                nc.vector.tensor_copy(pT_sb[:cn, t, :], pst[:cn, :128])
            ps_ctx = psump.tile([128, 512], f32, tag="p")
            for t, (c0, cn) in enumerate(kchunks):
                nc.tensor.matmul(
                    ps_ctx[:, :VD],
                    lhsT=pT_sb[:cn, t, :],
                    rhs=v_sb[:cn, t, :],
                    start=(t == 0),
                    stop=(t == len(kchunks) - 1),
                )
            ctx_sb = consts.tile([128, VD], f32)
            nc.vector.tensor_scalar_mul(ctx_sb, ps_ctx[:, :VD], recip[:, 0:1])
            nc.sync.dma_start(out=ctx_out[:, :], in_=ctx_sb)

    nc.compile()
    return nc


def _ensure_ntff_hook_module():
    """bass_utils' trace path (BASS_TRACE=1 under axon) imports
    antenv.axon_hooks unconditionally; some images lack that submodule.
    Provide a no-op implementation so tracing degrades instead of
    crashing.  Never overwrites a real or already-installed module."""
    import sys
    import types

    try:
        import antenv.axon_hooks  # noqa: F401
        return
    except ImportError:
        pass
    mod = types.ModuleType("antenv.axon_hooks")
    state = {"hook": None}
    mod.set_axon_ntff_profile_hook = lambda h: state.__setitem__("hook", h)
    mod.get_axon_ntff_profile_hook = lambda: state["hook"]
    sys.modules["antenv.axon_hooks"] = mod


def _kernel_numpy(Q, K, V, key_pad_mask, Wq, bq, Wk, bk, wV, bV):
    """Reference-equivalent numpy fallback for degenerate inputs."""
    mq = np.einsum("bqd,ed->bqe", Q, Wq) + bq
    mk = np.einsum("bkd,ed->bke", K, Wk) + bk
    scores = np.einsum(
        "bqke,e->bqk", np.tanh(mq[:, :, None, :] + mk[:, None, :, :]), wV.reshape(-1)
    ) + bV.reshape(-1)[0]
    scores = np.where(key_pad_mask[:, None, :], NEG, scores).astype(np.float32)
    e = np.exp(scores - scores.max(-1, keepdims=True))
    w = (e / e.sum(-1, keepdims=True)).astype(np.float32)
    ctx = np.einsum("bqk,bkv->bqv", w, V).astype(np.float32)
    return ctx, w


def kernel(Q, K, V, key_pad_mask, Wq, bq, Wk, bk, wV, bV):
    global LAST_RESULTS
    _ensure_ntff_hook_module()
    from concourse.bass_utils import run_bass_kernel_spmd

    Q = np.asarray(Q, np.float32)
    K = np.asarray(K, np.float32)
    V = np.asarray(V, np.float32)
    key_pad_mask = np.asarray(key_pad_mask, bool)
    Wq = np.asarray(Wq, np.float32)
    bq = np.asarray(bq, np.float32)
    Wk = np.asarray(Wk, np.float32)
    bk = np.asarray(bk, np.float32)
    wV = np.asarray(wV, np.float32).reshape(-1)  # [QD]
    bV = np.asarray(bV, np.float32).reshape(-1)  # [1]

    if Q.shape != (B, QL, QD) or K.shape != (B, KL, KD) or V.shape != (B, KL, VD):
        return _kernel_numpy(Q, K, V, key_pad_mask, Wq, bq, Wk, bk, wV, bV)

    valid = [np.nonzero(~key_pad_mask[b])[0] for b in range(B)]
    counts = [len(v) for v in valid]
    if min(counts) == 0:
        # a fully-masked batch makes the reference softmax uniform over all
        # keys; the compacted device path cannot represent that.
        return _kernel_numpy(Q, K, V, key_pad_mask, Wq, bq, Wk, bk, wV, bV)
    maxc = max(counts)
    KC = max(512, -(-maxc // 8) * 8)

    key = (KC,)
    if key not in _prog_cache:
        _prog_cache[key] = _build(KC)
    nc = _prog_cache[key]

    NE = QD // 128
    wqt = np.ascontiguousarray(Wq.T)  # [d, e]
    wkt = np.ascontiguousarray(Wk.T.astype(np.float16))  # [d, e]
    bq2 = np.ascontiguousarray(bq.reshape(NE, 128).T)  # [128, NE]
    bk2 = np.ascontiguousarray(bk.reshape(NE, 128).T)
    # wvr[p, c, m] = wV[c*128 + p]
    MW = 128 if _BF16_REDUCE else 1
    wvr = np.ascontiguousarray(np.broadcast_to(wV.reshape(NE, 128).T[:, :, None], (128, NE, MW)))
    if _BF16_REDUCE:
        import ml_dtypes
        wvr = np.ascontiguousarray(wvr.astype(ml_dtypes.bfloat16))
    identity = np.eye(128, dtype=np.float32)

    in_maps = []
    for c in range(N_CORES):
        b, half = divmod(c, 2)
        idx = valid[b]
        cnt = counts[b]
        qt_c = np.ascontiguousarray(Q[b, half * QS : (half + 1) * QS, :].T)  # [QD, QS]
        kt_c = np.zeros((KD, KC), np.float16)
        kt_c[:, :cnt] = K[b, idx, :].T.astype(np.float16)
        vc_c = np.zeros((KC, VD), np.float32)
        vc_c[:cnt] = V[b, idx, :]
        maskv = np.full((1, KC), NEG, np.float32)
        maskv[0, :cnt] = bV[0]
        in_maps.append(
            {
                "qt": qt_c,
                "kt": kt_c,
                "vc": vc_c,
                "wqt": wqt,
                "wkt": wkt,
                "bq2": bq2,
                "bk2": bk2,
                "wvr": wvr,
                "maskv": maskv,
                "ident": identity,
            }
        )

    res = run_bass_kernel_spmd(nc, in_maps, core_ids=list(range(N_CORES)))
    LAST_RESULTS = res

    attn_ctx = np.zeros((B, QL, VD), np.float32)
    attn_w = np.zeros((B, QL, KL), np.float32)
    for c in range(N_CORES):
        b, half = divmod(c, 2)
        idx = valid[b]
        cnt = counts[b]
        out = res.results[c]
        attn_ctx[b, half * QS : (half + 1) * QS, :] = out["ctx_out"]
        attn_w[b, half * QS : (half + 1) * QS][:, idx] = out["w_out"][:, :cnt]
    return attn_ctx, attn_w


# revision 24
# speedup vs baseline: 1.0099x; 1.0036x over previous
"""Additive (Bahdanau) attention TRN2 kernel.

Problem shapes (hardcoded): B=4, QL=256, KL=1024, QD=256, KD=512, VD=512.
Sharding: 8 cores = (batch, q-half); each core computes 128 query rows
against that batch's (compacted) keys.

Key insight: ~half the keys are masked by key_pad_mask, and the reference
gives those attention weight exactly 0.0 (exp(-1e12 - max) underflows).
So the host gathers only the valid keys per batch, the device computes
attention over the compacted key set, and the host scatters weights back.

Device algorithm per core (q = one of 128 query rows, e = 256 feature dim,
k = KC compacted keys):
  mKT[e, k] = (Wk @ K_c^T)[e, k] + bk[e]           (TensorE + DVE evac)
  mQT[e, q] = (Wq @ Q^T)[e, q] + bq[e]             (TensorE + DVE evac)
  per q:  T[e, k] = tanh(mKT[e, k] + mQT[e, q])    (ONE ScalarE activation
                                                    per 128-e chunk: the mQ
                                                    column is the per-partition
                                                    bias)
          scores_psum[:, k] = sum_e wV[e] T[e, k]  (TensorE matmul, stationary
                                                    operand = wV replicated to
                                                    128 cols -> every PSUM
                                                    partition holds the same
                                                    scores row)
          scores[q, k] = scores_psum[q, k] + mask[k]  (1-partition DVE add:
                                                    partition q reads its own
                                                    copy; mask = bV on valid
                                                    cols, -1e12 on padding)
  softmax over k (reduce_max negate -> Exp activation with accum_out sum ->
  reciprocal -> scale), ctx = weights @ V via PE transpose + matmul.
"""

import os

import numpy as np

B, QL, KL = 4, 256, 1024
QD, KD, VD = 256, 512, 512
N_CORES = 8
QS = QL // 2  # 128 queries per core
NEG = -1.0e12

# Reduce dtype for the big e-reduction + ctx matmuls: float32r streams at
# 1 col/cycle (vs 4 for float32) on the PE.
_F32R_REDUCE = os.environ.get("KERNEL_F32R", "1") == "1"
_BF16_REDUCE = os.environ.get("KERNEL_BF16", "0") == "1"

_prog_cache: dict[tuple, object] = {}

# Stash of the most recent BassKernelResults (for test harnesses to read
# exec_time_ns when BASS_TRACE=1).
LAST_RESULTS = None


def _build(KC: int):
    import concourse.bacc as bacc
    from concourse import mybir
    from concourse.tile import TileContext

    f32 = mybir.dt.float32
    f16 = mybir.dt.float16
    f32r = mybir.dt.float32r if _F32R_REDUCE else mybir.dt.float32
    rdt = mybir.dt.bfloat16 if _BF16_REDUCE else f32r
    MW = 128 if _BF16_REDUCE else 1  # stationary-operand column count
    AF = mybir.ActivationFunctionType

    NE = QD // 128  # 2 e-chunks
    ND_Q = QD // 128  # 2 d-chunks for Wq
    ND_K = KD // 128  # 4 d-chunks for Wk
    # N-splits of KC for matmul moving operand (<=512 each)
    nsplits = [(0, min(KC, 512))]
    if KC > 512:
        nsplits.append((512, KC - 512))
    # k chunks (<=128) for the ctx matmul contraction
    kchunks = []
    k0 = 0
    while k0 < KC:
        kchunks.append((k0, min(128, KC - k0)))
        k0 += 128

    nc = bacc.Bacc()

    qt = nc.dram_tensor("qt", [QD, QS], f32r, kind="ExternalInput")
    kt = nc.dram_tensor("kt", [KD, KC], f16, kind="ExternalInput")
    vc = nc.dram_tensor("vc", [KC, VD], f32r, kind="ExternalInput")
    wqt = nc.dram_tensor("wqt", [QD, QD], f32r, kind="ExternalInput")  # Wq.T
    wkt = nc.dram_tensor("wkt", [KD, QD], f16, kind="ExternalInput")  # Wk.T
    bq2 = nc.dram_tensor("bq2", [128, NE], f32, kind="ExternalInput")
    bk2 = nc.dram_tensor("bk2", [128, NE], f32, kind="ExternalInput")
    wvr = nc.dram_tensor("wvr", [128, NE, MW], rdt, kind="ExternalInput")
    maskv = nc.dram_tensor("maskv", [1, KC], f32, kind="ExternalInput")
    ident = nc.dram_tensor("ident", [128, 128], f32, kind="ExternalInput")

    w_out = nc.dram_tensor("w_out", [QS, KC], f32, kind="ExternalOutput")
    ctx_out = nc.dram_tensor("ctx_out", [QS, VD], f32, kind="ExternalOutput")

    with TileContext(nc) as tc:
        with (
            tc.tile_pool(name="consts", bufs=1) as consts,
            tc.tile_pool(name="tanh", bufs=2) as tanhp,
            tc.tile_pool(name="stage", bufs=2) as stagep,
            tc.tile_pool(name="psum", bufs=4, space="PSUM") as psump,
        ):
            # ---- PE early start: a few dummy matmuls on a zeroed tile so
            # the PE sequencer's instruction fetch + pipeline startup happen
            # during the DMA phase instead of stalling the first real matmul.
            warm_sb = consts.tile([128, 16], f32)
            nc.vector.memset(warm_sb, 0.0)
            warm_ps = psump.tile([1, 16], f32, tag="p")
            for _ in range(4):
                nc.tensor.matmul(
                    warm_ps[0:1, :],
                    lhsT=warm_sb[:, 0:1].bitcast(f32r),
                    rhs=warm_sb[:, :].bitcast(f32r),
                    start=True, stop=True,
                )

            # ---- load constants / inputs ----
            # DMA issue costs ~0.6us per dma_start on a queue; the first
            # matmuls need kt+wkt, so those issue first, split across the
            # sync and gpsimd queues.  V / identity / mask are not needed
            # until much later and issue lazily.
            # kt gates the first mapped_K matmuls; its last chunk goes at
            # the head of the scalar queue while chunks 0-2 stream on sync,
            # so all four arrive ~in parallel.  wkt rides gpsimd.
            kt_sb = consts.tile([128, ND_K, KC], f16)
            wkt_sb = consts.tile([128, ND_K, QD], f16)
            nc.scalar.dma_start(
                out=kt_sb[:, ND_K - 1, :], in_=kt[(ND_K - 1) * 128 :, :]
            )
            for c in range(ND_K - 1):
                nc.sync.dma_start(out=kt_sb[:, c, :], in_=kt[c * 128 : (c + 1) * 128, :])
            for c in range(ND_K):
                nc.gpsimd.dma_start(out=wkt_sb[:, c, :], in_=wkt[c * 128 : (c + 1) * 128, :])
            qt_sb = consts.tile([128, ND_Q, QS], f32r)
            wqt_sb = consts.tile([128, ND_Q, QD], f32r)
            for c in range(ND_Q):
                nc.scalar.dma_start(out=qt_sb[:, c, :], in_=qt[c * 128 : (c + 1) * 128, :])
                nc.scalar.dma_start(out=wqt_sb[:, c, :], in_=wqt[c * 128 : (c + 1) * 128, :])
            bq_sb = consts.tile([128, NE], f32)
            nc.scalar.dma_start(out=bq_sb, in_=bq2[:, :])
            bk_sb = consts.tile([128, NE], f32)
            nc.gpsimd.dma_start(out=bk_sb, in_=bk2[:, :])
            wv_sb = consts.tile([128, NE, MW], rdt)
            nc.gpsimd.dma_start(out=wv_sb, in_=wvr[:, :, :])
            mask_bc = consts.tile([128, KC], f32)
            nc.sync.dma_start(out=mask_bc, in_=maskv[0:1, :].to_broadcast([128, KC]))
            ident_sb = consts.tile([128, 128], f32)
            v_sb = consts.tile([128, len(kchunks), VD], f32r)

            # ---- mapped_Q^T [e, q] and mapped_K^T [e, k] (f32r matmuls) ----
            mkt_sb = consts.tile([128, NE, KC], f32)
            mqt_sb = consts.tile([128, NE, QS], f32)

            def emit_mkt(ce):
                for n0, nn in nsplits:
                    ps = psump.tile([128, 512], f32, tag="p", name=f"psk{ce}_{n0}")
                    for cd in range(ND_K):
                        nc.tensor.matmul(
                            ps[:, :nn],
                            lhsT=wkt_sb[:, cd, ce * 128 : (ce + 1) * 128],
                            rhs=kt_sb[:, cd, n0 : n0 + nn],
                            start=(cd == 0),
                            stop=(cd == ND_K - 1),
                        )
                    nc.vector.tensor_scalar_add(
                        mkt_sb[:, ce, n0 : n0 + nn], ps[:, :nn], bk_sb[:, ce : ce + 1]
                    )

            def emit_mqt(ce):
                ps = psump.tile([128, 512], f32, tag="p", name=f"psq{ce}")
                for cd in range(ND_Q):
                    nc.tensor.matmul(
                        ps[:, :QS],
                        lhsT=wqt_sb[:, cd, ce * 128 : (ce + 1) * 128],
                        rhs=qt_sb[:, cd, :],
                        start=(cd == 0),
                        stop=(cd == ND_Q - 1),
                    )
                nc.vector.tensor_scalar_add(mqt_sb[:, ce, :], ps[:, :QS], bq_sb[:, ce : ce + 1])

            # first ACTIVATE only needs mkt[ce0] and mqt[ce0]; mkt[ce1]
            # overlaps with the start of the main loop (PE has enough slack
            # per query to absorb it).
            emit_mkt(0)
            emit_mqt(0)
            emit_mqt(1)
            emit_mkt(1)

            # ---- main loop: scores, 2 queries per group ----
            # tanh tiles are grouped 2 queries wide so the ScalarE pays the
            # pool-reuse wait once per 2 ACTIVATEs, and the wv stationary
            # operand is reloaded once per (group, e-chunk) instead of per
            # matmul.  Each q's scores row lands on PSUM partition 0 (M=1
            # matmul); DVE copies it into a partition-0 staging tile of G
            # rows which is scatter-DMA'd (SBUF->SBUF) into [q, k] layout.
            G2 = 8   # queries per tanh tile group
            GP = 4   # queries whose matmuls run per stationary-operand pass
            G = 8    # queries per scatter chunk
            scores_sb = consts.tile([128, KC], f32)
            stage = None
            for g in range(QS // G2):
                ths = []
                for ce in range(NE):
                    th_grp = tanhp.tile([128, G2, KC], rdt, tag=f"tanh{ce}", name=f"th{g}_{ce}")
                    ths.append(th_grp)
                for j in range(G2):
                    q = g * G2 + j
                    for ce in range(NE):
                        nc.scalar.activation(
                            ths[ce][:, j, :],
                            mkt_sb[:, ce, :],
                            AF.Tanh,
                            bias=mqt_sb[:, ce, q : q + 1],
                            scale=1.0,
                        )
                for jp in range(G2 // GP):
                    pss = []
                    for j2 in range(GP):
                        ps_j = psump.tile([MW, KC], f32, tag="p", name=f"ps{g}_{jp}_{j2}")
                        pss.append(ps_j)
                    for ce in range(NE):
                        for j2 in range(GP):
                            j = jp * GP + j2
                            for n0, nn in nsplits:
                                nc.tensor.matmul(
                                    pss[j2][0:MW, n0 : n0 + nn],
                                    lhsT=wv_sb[:, ce, :],
                                    rhs=ths[ce][:, j, n0 : n0 + nn],
                                    start=(ce == 0),
                                    stop=(ce == NE - 1),
                                )
                    for j2 in range(GP):
                        q = g * G2 + jp * GP + j2
                        if stage is None:
                            stage = stagep.tile([1, G, KC], f32, tag="stage")
                        # evacuate + apply padding mask/bV in one op (all of
                        # these rows live on partition 0)
                        nc.vector.tensor_add(
                            stage[0:1, q % G, :], pss[j2][0:1, :], mask_bc[0:1, :]
                        )
                        if q % G == G - 1:
                            nc.sync.dma_start(
                                out=scores_sb[q - G + 1 : q + 1, :],
                                in_=stage[0:1, :, :],
                            )
                            stage = None

            # ---- softmax ----
            # no max-subtraction: |scores| <= sum|wV| + |bV| < 26 (tanh in
            # [-1,1]), so exp cannot overflow fp32; padded columns are -1e12
            # and exp to exactly 0.  Matches the reference softmax exactly up
            # to fp32 rounding.
            p_sb = consts.tile([128, KC], f32)
            sum_sb = consts.tile([128, 1], f32)
            nc.scalar.activation(
                p_sb, scores_sb, AF.Exp, bias=0.0, scale=1.0, accum_out=sum_sb
            )
            recip = consts.tile([128, 1], f32)
            nc.vector.reciprocal(recip, sum_sb)
            wout_sb = consts.tile([128, KC], f32)
            nc.vector.tensor_scalar_mul(wout_sb, p_sb, recip[:, 0:1])
            nc.sync.dma_start(out=w_out[:, :], in_=wout_sb)

            # ---- ctx = softmax(scores) @ V ----
            nc.gpsimd.dma_start(out=ident_sb, in_=ident[:, :])
            for t, (c0, cn) in enumerate(kchunks):
                nc.gpsimd.dma_start(out=v_sb[:cn, t, :], in_=vc[c0 : c0 + cn, :])
            pT_sb = consts.tile([128, len(kchunks), 128], f32r)
            for t, (c0, cn) in enumerate(kchunks):
                pst = psump.tile([128, 512], f32, tag="p")
                nc.tensor.transpose(pst[:cn, :128], p_sb[:, c0 : c0 + cn], ident_sb)
                nc.vector.tensor_copy(pT_sb[:cn, t, :], pst[:cn, :128])
            ps_ctx = psump.tile([128, 512], f32, tag="p")
            for t, (c0, cn) in enumerate(kchunks):
                nc.tensor.matmul(
                    ps_ctx[:, :VD],
                    lhsT=pT_sb[:cn, t, :],
                    rhs=v_sb[:cn, t, :],
                    start=(t == 0),
                    stop=(t == len(kchunks) - 1),
                )
            ctx_sb = consts.tile([128, VD], f32)
            nc.vector.tensor_scalar_mul(ctx_sb, ps_ctx[:, :VD], recip[:, 0:1])
            nc.sync.dma_start(out=ctx_out[:, :], in_=ctx_sb)

    nc.compile()
    return nc


def _ensure_ntff_hook_module():
    """bass_utils' trace path (BASS_TRACE=1 under axon) imports
    antenv.axon_hooks unconditionally; some images lack that submodule.
    Provide a no-op implementation so tracing degrades instead of
    crashing.  Never overwrites a real or already-installed module."""
    import sys
    import types

    try:
        import antenv.axon_hooks  # noqa: F401
        return
    except ImportError:
        pass
    mod = types.ModuleType("antenv.axon_hooks")
    state = {"hook": None}
    mod.set_axon_ntff_profile_hook = lambda h: state.__setitem__("hook", h)
    mod.get_axon_ntff_profile_hook = lambda: state["hook"]
    sys.modules["antenv.axon_hooks"] = mod


def _kernel_numpy(Q, K, V, key_pad_mask, Wq, bq, Wk, bk, wV, bV):
    """Reference-equivalent numpy fallback for degenerate inputs."""
    mq = np.einsum("bqd,ed->bqe", Q, Wq) + bq
    mk = np.einsum("bkd,ed->bke", K, Wk) + bk
    scores = np.einsum(
        "bqke,e->bqk", np.tanh(mq[:, :, None, :] + mk[:, None, :, :]), wV.reshape(-1)
    ) + bV.reshape(-1)[0]
    scores = np.where(key_pad_mask[:, None, :], NEG, scores).astype(np.float32)
    e = np.exp(scores - scores.max(-1, keepdims=True))
    w = (e / e.sum(-1, keepdims=True)).astype(np.float32)
    ctx = np.einsum("bqk,bkv->bqv", w, V).astype(np.float32)
    return ctx, w


def kernel(Q, K, V, key_pad_mask, Wq, bq, Wk, bk, wV, bV):
    global LAST_RESULTS
    _ensure_ntff_hook_module()
    from concourse.bass_utils import run_bass_kernel_spmd

    Q = np.asarray(Q, np.float32)
    K = np.asarray(K, np.float32)
    V = np.asarray(V, np.float32)
    key_pad_mask = np.asarray(key_pad_mask, bool)
    Wq = np.asarray(Wq, np.float32)
    bq = np.asarray(bq, np.float32)
    Wk = np.asarray(Wk, np.float32)
    bk = np.asarray(bk, np.float32)
    wV = np.asarray(wV, np.float32).reshape(-1)  # [QD]
    bV = np.asarray(bV, np.float32).reshape(-1)  # [1]

    if Q.shape != (B, QL, QD) or K.shape != (B, KL, KD) or V.shape != (B, KL, VD):
        return _kernel_numpy(Q, K, V, key_pad_mask, Wq, bq, Wk, bk, wV, bV)

    valid = [np.nonzero(~key_pad_mask[b])[0] for b in range(B)]
    counts = [len(v) for v in valid]
    if min(counts) == 0:
        # a fully-masked batch makes the reference softmax uniform over all
        # keys; the compacted device path cannot represent that.
        return _kernel_numpy(Q, K, V, key_pad_mask, Wq, bq, Wk, bk, wV, bV)
    maxc = max(counts)
    KC = max(512, -(-maxc // 8) * 8)

    key = (KC,)
    if key not in _prog_cache:
        _prog_cache[key] = _build(KC)
    nc = _prog_cache[key]

    NE = QD // 128
    wqt = np.ascontiguousarray(Wq.T)  # [d, e]
    wkt = np.ascontiguousarray(Wk.T.astype(np.float16))  # [d, e]
    bq2 = np.ascontiguousarray(bq.reshape(NE, 128).T)  # [128, NE]
    bk2 = np.ascontiguousarray(bk.reshape(NE, 128).T)
    # wvr[p, c, m] = wV[c*128 + p]
    MW = 128 if _BF16_REDUCE else 1
    wvr = np.ascontiguousarray(np.broadcast_to(wV.reshape(NE, 128).T[:, :, None], (128, NE, MW)))
    if _BF16_REDUCE:
        import ml_dtypes
        wvr = np.ascontiguousarray(wvr.astype(ml_dtypes.bfloat16))
    identity = np.eye(128, dtype=np.float32)

    in_maps = []
    for c in range(N_CORES):
        b, half = divmod(c, 2)
        idx = valid[b]
        cnt = counts[b]
        qt_c = np.ascontiguousarray(Q[b, half * QS : (half + 1) * QS, :].T)  # [QD, QS]
        kt_c = np.zeros((KD, KC), np.float16)
        kt_c[:, :cnt] = K[b, idx, :].T.astype(np.float16)
        vc_c = np.zeros((KC, VD), np.float32)
        vc_c[:cnt] = V[b, idx, :]
        maskv = np.full((1, KC), NEG, np.float32)
        maskv[0, :cnt] = bV[0]
        in_maps.append(
            {
                "qt": qt_c,
                "kt": kt_c,
                "vc": vc_c,
                "wqt": wqt,
                "wkt": wkt,
                "bq2": bq2,
                "bk2": bk2,
                "wvr": wvr,
                "maskv": maskv,
                "ident": identity,
            }
        )

    res = run_bass_kernel_spmd(nc, in_maps, core_ids=list(range(N_CORES)))
    LAST_RESULTS = res

    attn_ctx = np.zeros((B, QL, VD), np.float32)
    attn_w = np.zeros((B, QL, KL), np.float32)
    for c in range(N_CORES):
        b, half = divmod(c, 2)
        idx = valid[b]
        cnt = counts[b]
        out = res.results[c]
        attn_ctx[b, half * QS : (half + 1) * QS, :] = out["ctx_out"]
        attn_w[b, half * QS : (half + 1) * QS][:, idx] = out["w_out"][:, :cnt]
    return attn_ctx, attn_w
